# revision 65
# baseline (speedup 1.0000x reference)
"""Trainium2 Bass kernel for batched multi-head attention.

Problem: softmax(q @ k^T / sqrt(64)) @ v with q,k,v [4, 16, 2048, 64] f32.
Sharding: batch*heads (64) split across 8 NeuronCores, 8 heads per core.

v2 design (engine-balanced, per core):
  - heads processed in pairs; Q,K PE-transposed into stacked [128, 2048]
    bf16 tiles (rows 0:64 = head A's d, rows 64:128 = head B's d) so the
    psum->SBUF staging copies use all 128 partitions.
  - exp split across engines: 11/16 of score tiles take exact Exp on the
    Scalar (ACT) engine, 5/16 take a Schraudolph bf16-bits approximation
    on the Vector (DVE) engine (one tensor_scalar: i16 = trunc(a*s + b),
    bits reinterpreted as bf16 ~= exp(s/8); rms err ~1.8% on those tiles,
    ~1% end-to-end after softmax-weight cancellation).
  - casts q,k->bf16 and V-augmentation run on GPSIMD (Pool) so DVE keeps
    headroom (Pool is SBUF-only).
  - PV with V stationary [128, 65] (ones column produces denominators);
    output copied psum->SBUF as bf16 on ACT (Copy shares the act table
    with Exp), PE-transposed back, normalized on DVE with a reciprocal +
    stride-0-broadcast multiply, DMA'd out as f32.
"""

import os
import sys
from contextlib import ExitStack

import numpy as np

for _p in (
    "/root/.axon_site",
    "/root/.axon_site/_ro/trn_rl_repo",
    "/root/.axon_site/_ro/pypackages",
    "/opt/trn_rl_repo",
):
    if os.path.isdir(_p) and _p not in sys.path:
        sys.path.append(_p)

import concourse.bass as bass  # noqa: E402
import concourse.tile as tile  # noqa: E402
from concourse import bacc, mybir  # noqa: E402
from concourse.bass import ds, ts  # noqa: E402
from concourse.bass_utils import run_bass_kernel_spmd  # noqa: E402
from concourse.masks import make_identity  # noqa: E402

N_CORES = 8
B, H, S, D = 4, 16, 2048, 64
HPC = (B * H) // N_CORES  # heads per core
SCALE = 1.0 / np.sqrt(np.float32(D)).astype(np.float32)

F32 = mybir.dt.float32
BF16 = mybir.dt.bfloat16
I16 = mybir.dt.int16

NT = S // 128  # 16 seq tiles of 128
NCH = 2  # q chunks of 1024
CH = S // NCH

# Schraudolph exp in bf16-bit domain: i16 = trunc(A*s + B) viewed as bf16
# approximates exp(s/8) (1/8 softmax scale folded into A).
SCH_A = float(128.0 * 1.4426950408889634 / 8.0)
SCH_B = 16249.125
# k-tiles handled by DVE-Schraudolph (4 of 16, evenly spread; ACT has
# just enough headroom for 12/16 and exact exp is more accurate)
DVE_KTI = frozenset((2, 5, 8, 11))


def _build_nc():
    nc = bacc.Bacc(
        "TRN2", target_bir_lowering=False, debug=False, num_devices=N_CORES
    )
    q = nc.declare_dram_parameter("q", [HPC, S, D], F32, isOutput=False).ap()
    k = nc.declare_dram_parameter("k", [HPC, S, D], F32, isOutput=False).ap()
    v = nc.declare_dram_parameter("v", [HPC, S, D], F32, isOutput=False).ap()
    out = nc.declare_dram_parameter("out", [HPC, S, D], F32, isOutput=True).ap()

    with tile.TileContext(nc) as tc, ExitStack() as ctx:
        consts = ctx.enter_context(tc.tile_pool(name="consts", bufs=1))
        id_bf = consts.tile([128, 128], BF16)
        make_identity(nc, id_bf[:])

        ld = ctx.enter_context(tc.tile_pool(name="ld", bufs=2))
        cast = ctx.enter_context(tc.tile_pool(name="cast", bufs=2))
        vp = ctx.enter_context(tc.tile_pool(name="vp", bufs=4))
        qkt = ctx.enter_context(tc.tile_pool(name="qkt", bufs=2))
        ptp = ctx.enter_context(tc.tile_pool(name="ptp", bufs=6))
        obp = ctx.enter_context(tc.tile_pool(name="obp", bufs=3))
        otp = ctx.enter_context(tc.tile_pool(name="otp", bufs=4))
        ofp = ctx.enter_context(tc.tile_pool(name="ofp", bufs=3))
        rp = ctx.enter_context(tc.tile_pool(name="rp", bufs=3))

        spsum = ctx.enter_context(tc.tile_pool(name="spsum", bufs=2, space="PSUM"))
        opsum = ctx.enter_context(tc.tile_pool(name="opsum", bufs=2, space="PSUM"))

        pending = []
        staged = []

        def tail_transpose(h_, q0_, ob_, nb=CH // 128):
            # ob_ holds the un-normalized out^T [80, nb*128] in SBUF bf16
            # (row 64 = ones-column denominators). DMA-XBAR transpose to
            # [q, d] blocks in SBUF (SP engine).
            ot = otp.tile([128, nb, 80], BF16, tag=f"ot{nb}")
            nc.sync.dma_start_transpose(ot[:], ob_[:])
            return (h_, q0_, ot, nb)

        def tail_store(h_, t0_, ot_, nb):
            # ot row 65 carries the pre-computed denominator reciprocals
            # (written into ob before the transpose), so the tail is a pure
            # Pool broadcast-multiply + SP store — no DVE/ACT instruction
            # can convoy behind the (possibly slow) XBAR DMA. ot block
            # (p, b) holds q = p*16 + t0 + b, so the store writes nb
            # consecutive rows per partition (2KB contiguous runs).
            of = ofp.tile([128, nb, D], F32, tag="of")
            nc.gpsimd.tensor_tensor(
                out=of[:],
                in0=ot_[:, :, 0:D],
                in1=ot_[:, :, D : D + 1].broadcast_to((128, nb, D)),
                op=mybir.AluOpType.mult,
            )
            nc.sync.dma_start(
                out[h_].rearrange("(p t) d -> p t d", p=128)[:, ds(t0_, nb), :],
                of[:],
            )

        def prologue_loads(pair, first=False):
            # Input loads spread across engines: the cost model charges
            # transfer time to the issuing engine's queue, so one engine
            # can't carry all ~76us of input DMA. Pool's first DMA pays an
            # ~11us engine-init delay, so pair 0 avoids Pool on the
            # critical path (qf/vf on SP, kf on ACT).
            tiles = []
            for h in (2 * pair, 2 * pair + 1):
                qf = ld.tile([128, NT, D], F32, tag="qf", name=f"qf{h}")
                kf = ld.tile([128, NT, D], F32, tag="kf", name=f"kf{h}")
                vf = ld.tile([128, NT, D], F32, tag="vf", name=f"vf{h}")
                tiles.append([qf, kf, vf])
            return tiles

        def prologue_rest(pair, tiles, first=False):
            # bf16 casts on Pool — in-order BEHIND the loads they consume on
            # the same engine, so the tile scheduler can't hoist them into
            # another engine's stream where their load-wait would block
            # critical exps (pair 0 uses DVE casts: Pool pays an ~11us init
            # delay on its first instruction and DVE is empty at startup).
            # Q,K transposed via the DMA XBAR (one instruction per
            # head-tensor), head-pair stacked: rows 0:64 = head A, rows
            # 64:128 = head B. out[p, t, c] = in[c, t*64+p] gives the
            # [d, seq] layout directly.
            # The XBAR transpose needs a full-128-partition contiguous
            # destination (hardware constraint, not modeled by CoreSim), so
            # both heads cast into ONE combined [128, NT, 2, 64] tile whose
            # (head, d) free packing transposes straight into the stacked
            # qt2/kt2 rows: out[p=hi*64+d, t, c] = in[c, t, hi, d].
            qt2 = qkt.tile([128, NT, 128], BF16, tag="qt", name=f"qt2_{pair}")
            kt2 = qkt.tile([128, NT, 128], BF16, tag="kt", name=f"kt2_{pair}")
            qb2 = cast.tile([128, NT, 2, D], BF16, tag="qb", name=f"qb2_{pair}")
            kb2 = cast.tile([128, NT, 2, D], BF16, tag="kb", name=f"kb2_{pair}")
            vaugs = []
            for hi, (qf, kf, vf) in enumerate(tiles):
                h = 2 * pair + hi
                # partition = seq//16 so each load is one 4KB-contiguous
                # descriptor per partition — 256B runs pay a 2x DMA latency
                # penalty and all transfers serialize on the DMA engines.
                # The permuted k/q orderings flow consistently through
                # QK -> exp -> PV -> store (k-class kti = k%16).
                nc.sync.dma_start(
                    kf[:], k[h].rearrange("(p t) d -> p t d", p=128)
                )
                nc.gpsimd.dma_start(
                    qf[:], q[h].rearrange("(p t) d -> p t d", p=128)
                )
                nc.gpsimd.tensor_copy(kb2[:, :, hi, :], kf[:])
                nc.gpsimd.tensor_copy(qb2[:, :, hi, :], qf[:])
                nc.gpsimd.dma_start(vf[:], v[h].rearrange("(p t) d -> p t d", p=128))
                # cols D:80 all 1.0 — col D is the real ones-column for the
                # softmax denominators; 65:80 pad the PV output to 80
                # partitions so the tail's DMA transpose has p_dim % 16 == 0
                vaug = vp.tile([128, NT, 80], BF16, tag="vaug")
                nc.gpsimd.memset(vaug[:, :, D:80], 1.0)
                nc.gpsimd.tensor_copy(vaug[:, :, 0:D], vf[:])
                vaugs.append(vaug)
            nc.sync.dma_start_transpose(
                kt2[:], kb2[:].rearrange("p t h d -> p (t h d)")
            )
            nc.sync.dma_start_transpose(
                qt2[:], qb2[:].rearrange("p t h d -> p (t h d)")
            )
            return qt2, kt2, vaugs

        def head_attention(h, rb, qt2, kt2, vaug, last=False):
                # Both q-chunks of this head run interleaved kti-by-kti, and
                # PV lags QK by one kti, so every semaphore edge (QK->exp,
                # exp->PV, exp->ss-buffer-reuse) has multi-microsecond slack
                # and the PE never stalls on a just-fired semaphore.
                # padded to 80 partitions so the tail's DMA transpose has
                # p_dim % 16 == 0; rows 65:80 are never written or read
                # beyond the transpose (CoreSim zero-fills, HW has garbage
                # that lands in ot[:, :, 65:80], which is never read).
                pos = [
                    opsum.tile([80, CH], F32, tag="po", name=f"po{c}")
                    for c in range(NCH)
                ]
                prev_pts = None

                def emit_pv(kti, pts):
                    for c in range(NCH):
                        for j in range(2):
                            nc.tensor.matmul(
                                pos[c][:, ts(j, 512)],
                                lhsT=vaug[:, kti],
                                rhs=pts[c][:, ts(j, 512)],
                                start=(kti == 0),
                                stop=(kti == NT - 1),
                            )

                for kti in range(NT):
                    sss = []
                    for c in range(NCH):
                        ss = spsum.tile([128, CH], F32, tag="ss")
                        for j in range(2):
                            nc.tensor.matmul(
                                ss[:, ts(j, 512)],
                                lhsT=kt2[rb : rb + 64, kti],
                                rhs=qt2[rb : rb + 64, ds(c * (CH // 128) + j * 4, 4)],
                                start=True,
                                stop=True,
                            )
                        sss.append(ss)
                    pts = []
                    for c in range(NCH):
                        pt = ptp.tile([128, CH], BF16, tag="pt")
                        if kti in DVE_KTI:
                            nc.vector.tensor_scalar(
                                out=pt[:].bitcast(I16),
                                in0=sss[c][:],
                                scalar1=SCH_A,
                                scalar2=SCH_B,
                                op0=mybir.AluOpType.mult,
                                op1=mybir.AluOpType.add,
                            )
                        else:
                            nc.scalar.activation(
                                pt[:],
                                sss[c][:],
                                mybir.ActivationFunctionType.Exp,
                                bias=0.0,
                                scale=float(SCALE),
                            )
                        pts.append(pt)
                    if prev_pts is not None:
                        emit_pv(kti - 1, prev_pts)
                    prev_pts = pts
                    # previous head's output tails, staged mid-stream:
                    # transposes early (kti 2/4), normalize+store later
                    # (kti 8/10) once the XBAR results have long landed
                    if kti in (2, 4) and pending:
                        staged.append(tail_transpose(*pending.pop(0)))
                    if kti in (13, 15) and staged:
                        tail_store(*staged.pop(0))
                emit_pv(NT - 1, prev_pts)

                # chunk-end psum->SBUF copies, split across DVE and ACT so
                # the po buffers free up in parallel without stalling PE;
                # then DMA-XBAR transpose out^T -> [q, d] blocks in SBUF.
                # For the final head, split everything into 512-col halves
                # and drain immediately to shorten the end-of-kernel chain.
                if not last:
                    # both copies on DVE: ACT's in-order stream must stay
                    # pure exps — an ob copy there displaces the head's
                    # final exps and stalls PE ~1.2us per head
                    # row 65 of ob := 1/denominator, computed from the psum
                    # ones-column while it's still resident; it rides the
                    # XBAR transpose so the tail needs no reciprocal
                    for c in range(NCH):
                        ob = obp.tile([80, CH], BF16, tag="ob")
                        nc.vector.tensor_copy(ob[:], pos[c][:])
                        # overwrite the raw-denominator row in place (DVE
                        # partition starts must be quad-aligned; 64 is, 65
                        # is not)
                        with nc.allow_low_precision(
                            reason="bf16 1/denominator, ~0.4% quant is in budget"
                        ):
                            nc.vector.reciprocal(
                                ob[D : D + 1, :], pos[c][D : D + 1, :]
                            )
                        pending.append((h, c * (CH // 128), ob))
                else:
                    for c in range(NCH):
                        for half in range(2):
                            ob = obp.tile([80, CH // 2], BF16, tag="obh", bufs=4)
                            if half == 0:
                                nc.vector.tensor_copy(
                                    ob[:], pos[c][:, ts(half, CH // 2)]
                                )
                            else:
                                nc.scalar.copy(ob[:], pos[c][:, ts(half, CH // 2)])
                            with nc.allow_low_precision(
                                reason="bf16 1/denominator, ~0.4% quant ok"
                            ):
                                nc.vector.reciprocal(
                                    ob[D : D + 1, :],
                                    pos[c][D : D + 1, ts(half, CH // 2)],
                                )
                            tail_store(
                                *tail_transpose(
                                    h,
                                    c * (CH // 128) + half * (CH // 256),
                                    ob,
                                    CH // 256,
                                )
                            )

        pro = prologue_rest(0, prologue_loads(0, first=True), first=True)
        for pair in range(HPC // 2):
            heads = (2 * pair, 2 * pair + 1)
            qt2, kt2, vaugs = pro
            is_last_pair = pair + 1 >= HPC // 2
            head_attention(heads[0], 0, qt2, kt2, vaugs[0])
            if not is_last_pair:
                pro = prologue_rest(pair + 1, prologue_loads(pair + 1))
            head_attention(heads[1], 64, qt2, kt2, vaugs[1], last=is_last_pair)

        while pending:
            staged.append(tail_transpose(*pending.pop(0)))
        while staged:
            tail_store(*staged.pop(0))

    nc.finalize()
    return nc


class _Runner:
    """Persistent compiled SPMD executor (mirrors bass2jax.run_bass_via_pjrt's
    multi-core path, but keeps the jitted callable so repeated calls reuse the
    compiled NEFF)."""

    def __init__(self):
        import jax
        from concourse import bass2jax
        from jax.experimental.shard_map import shard_map
        from jax.sharding import Mesh, PartitionSpec

        try:
            jax.config.update("jax_compilation_cache_dir", "/tmp/jax_bass_cache")
            jax.config.update("jax_persistent_cache_min_compile_time_secs", 10)
        except Exception:
            pass
        bass2jax.install_neuronx_cc_hook()
        self.jax = jax
        nc = _build_nc()
        self.nc = nc

        in_names = []
        out_names = []
        out_avals = []
        for alloc in nc.m.functions[0].allocations:
            if not isinstance(alloc, mybir.MemoryLocationSet):
                continue
            name = alloc.memorylocations[0].name
            if alloc.kind == "ExternalInput":
                in_names.append(name)
            elif alloc.kind == "ExternalOutput":
                out_names.append(name)
                out_avals.append(
                    jax.core.ShapedArray(
                        tuple(alloc.tensor_shape), mybir.dt.np(alloc.dtype)
                    )
                )
        assert nc.dbg_addr is None
        partition_name = (
            nc.partition_id_tensor.name if nc.partition_id_tensor else None
        )
        # partition_id is an ExternalInput allocation but is supplied by
        # PartitionIdOp, not by the caller — drop it from the caller list.
        if partition_name is not None and partition_name in in_names:
            in_names.remove(partition_name)
        self.in_names = list(in_names)
        self.out_names = list(out_names)
        self.out_avals = out_avals
        all_in_names = in_names + out_names
        if partition_name is not None:
            all_in_names = all_in_names + [partition_name]

        def _body(*args):
            operands = list(args)
            if partition_name is not None:
                operands.append(bass2jax.partition_id_tensor())
            outs = bass2jax._bass_exec_p.bind(
                *operands,
                out_avals=tuple(out_avals),
                in_names=tuple(all_in_names),
                out_names=tuple(out_names),
                lowering_input_output_aliases=(),
                sim_require_finite=True,
                sim_require_nnan=True,
                nc=nc,
            )
            return tuple(outs)

        devices = jax.devices()[:N_CORES]
        assert len(devices) == N_CORES
        mesh = Mesh(np.asarray(devices), ("core",))
        n_args = len(in_names) + len(out_names)
        self._fn = jax.jit(
            shard_map(
                _body,
                mesh=mesh,
                in_specs=(PartitionSpec("core"),) * n_args,
                out_specs=(PartitionSpec("core"),) * len(out_names),
                check_rep=False,
            ),
            keep_unused=True,
        )
        from jax.sharding import NamedSharding

        self._sharding = NamedSharding(mesh, PartitionSpec("core"))
        # Device-resident zero "output-init" operands: the custom call never
        # writes its operands (no aliasing), so these are reusable every call.
        self._zeros = [
            jax.device_put(
                np.zeros((N_CORES * a.shape[0], *a.shape[1:]), a.dtype),
                self._sharding,
            )
            for a in out_avals
        ]

    def device_put(self, arr):
        return self.jax.device_put(arr, self._sharding)

    def __call__(self, concat_inputs):
        """concat_inputs: dict name -> np/jax array of shape [8*HPC, ...]."""
        args = [concat_inputs[n] for n in self.in_names] + list(self._zeros)
        outs = self._fn(*args)
        return {n: outs[i] for i, n in enumerate(self.out_names)}


_RUNNER = None


def _get_runner():
    global _RUNNER
    if _RUNNER is None:
        _RUNNER = _Runner()
    return _RUNNER


def _concat_inputs(q, k, v):
    qr = np.ascontiguousarray(np.asarray(q, dtype=np.float32)).reshape(B * H, S, D)
    kr = np.ascontiguousarray(np.asarray(k, dtype=np.float32)).reshape(B * H, S, D)
    vr = np.ascontiguousarray(np.asarray(v, dtype=np.float32)).reshape(B * H, S, D)
    return {"q": qr, "k": kr, "v": vr}


def run(q, k, v):
    runner = _get_runner()
    outs = runner(_concat_inputs(q, k, v))
    return np.asarray(outs["out"]).reshape(B, H, S, D)


def bench(q, k, v, iters=20):
    """Time back-to-back executions with device-resident inputs.
    Returns (per_call_seconds_estimate, out)."""
    import time

    runner = _get_runner()
    jax = runner.jax
    ins = _concat_inputs(q, k, v)
    dev_ins = {n: runner.device_put(a) for n, a in ins.items()}
    out = runner(dev_ins)
    jax.block_until_ready(out)

    def timed(n):
        t0 = time.perf_counter()
        o = None
        for _ in range(n):
            o = runner(dev_ins)
        jax.block_until_ready(o)
        return time.perf_counter() - t0

    timed(2)
    n1, n2 = max(2, iters // 4), iters
    t1 = min(timed(n1) for _ in range(2))
    t2 = min(timed(n2) for _ in range(2))
    slope = (t2 - t1) / (n2 - n1)
    return slope, np.asarray(out["out"]).reshape(B, H, S, D)


def kernel(q, k, v):
    return run(q, k, v)
